# revision 59
# baseline (speedup 1.0000x reference)
"""Trainium2 Bass kernel for Performer-style (FAVOR+) causal linear attention.

Reference computation (per batch b=1, heads h=16, seq s=2048, d=64, r=64):
  qh = split_heads((q @ wq + bq) * d^-0.25)     kh likewise, vh = split_heads(v @ wv + bv)
  q' = (1/sqrt(d)) * exp(qh @ wg - 0.5*||qh||^2)   k' likewise
  attn[s] = (q'_s . sum_{j<=s} k'_j v_j^T) / (eps + q'_s . sum_{j<=s} k'_j)
  out = merge_heads(attn) @ wc + bc

Key simplifications:
  - wg is orthogonal (64x64 from QR), so ||qh||^2 == ||qh @ wg||^2. Folding
    wg into the projection weights (wqg = norm * wq @ blockdiag(wg)) means
    the kernel only computes qhg = q @ wqg and
    q' = exp(qhg) * exp(-0.5*sum_r qhg_r^2) / sqrt(d).
  - The causal scan is de-serialized: each chunk's state is an independent
    single matmul (both heads packed via the augmented-value layout);
    prefix states accumulate on the vector engine, with an fp16 copy on
    GpSimd feeding the inter-chunk matmuls.

Sharding: 2 heads per core (16 heads over 8 cores). Each core gets full
fp16 q/k/v (transposed) + its 128-column weight slices, computes its heads'
attention, projects through its 128-row slice of wc, and returns a
(2048, 1024) fp16 partial. The host sums the 8 partials and adds wc_b.
"""

import sys

if "/opt/trn_rl_repo" not in sys.path:
    sys.path.insert(0, "/opt/trn_rl_repo")

import math
from contextlib import ExitStack

import numpy as np

D_MODEL = 1024
N_HEADS = 16
D = 64  # head depth
R = 64  # kernel features (= D, wg orthogonal)
S = 2048
N_CORES = 8
HPC = N_HEADS // N_CORES  # heads per core = 2
CW = HPC * D  # per-core channel width = 128
P = 128
ST = 512  # projection s-tile width
NST = S // ST  # 4
C = 128  # scan chunk
NCH = S // C  # 16
KT = D_MODEL // P  # 8 contraction tiles
W = D + 1  # augmented value width (v | 1)
W2 = HPC * W  # 130
NORM_D = float(D ** (-0.25))
LN_RSQRT_D = float(-0.5 * math.log(D))  # exp(x + this) = exp(x)/sqrt(d)

_CACHE = {}


def _build_bass():
    import concourse.bass as bass
    import concourse.mybir as mybir
    import concourse.tile as tile
    from concourse.bacc import Bacc

    f16 = mybir.dt.float16
    f32 = mybir.dt.float32
    AF = mybir.ActivationFunctionType
    Alu = mybir.AluOpType

    nc = Bacc(trn_type="TRN2")

    qT = nc.dram_tensor("qT", [D_MODEL, S], f16, kind="ExternalInput")
    kT = nc.dram_tensor("kT", [D_MODEL, S], f16, kind="ExternalInput")
    vT = nc.dram_tensor("vT", [D_MODEL, S], f16, kind="ExternalInput")
    # weights host-prearranged to [128, k*cw] so the DMA is flat
    wq = nc.dram_tensor("wq", [P, KT * CW], f16, kind="ExternalInput")
    wk = nc.dram_tensor("wk", [P, KT * CW], f16, kind="ExternalInput")
    wv = nc.dram_tensor("wv", [P, KT * CW], f16, kind="ExternalInput")
    # aux: [ident(128) | mask(128) | ng(64)] packed along free dim
    aux = nc.dram_tensor("aux", [P, 2 * P + R], f16, kind="ExternalInput")
    bqkv = nc.dram_tensor("bqkv", [CW, 3], f32, kind="ExternalInput")
    wc = nc.dram_tensor("wc", [CW, D_MODEL], f16, kind="ExternalInput")
    out = nc.dram_tensor("out", [S, D_MODEL], f16, kind="ExternalOutput")

    with tile.TileContext(nc) as tc, ExitStack() as ctx:
        # ---- constant / weight tiles (sync queue, ahead of the x stream) ----
        const = ctx.enter_context(tc.tile_pool(name="const", bufs=1))
        w_sb = {}
        for name, drt in (("wq", wq), ("wk", wk), ("wv", wv)):
            t = const.tile([P, KT * CW], f16, tag=name, name=f"wt_{name}")
            nc.sync.dma_start(t[:], drt[:, :])
            for k in range(KT):
                w_sb[(name, k)] = t[:, k * CW : (k + 1) * CW]
        b_all = const.tile([CW, 3], f32, tag="ball")
        nc.sync.dma_start(b_all[:], bqkv[:, :])
        b_sb = {"bq": b_all[:, 0:1], "bk": b_all[:, 1:2], "bv": b_all[:, 2:3]}
        aux_sb = const.tile([P, 2 * P + R], f16, tag="aux")
        nc.sync.dma_start(aux_sb[:], aux[:, :])
        id_sb = aux_sb[:, 0:P]
        mask_sb = aux_sb[:, P : 2 * P]
        ng_sb = aux_sb[:, 2 * P : 2 * P + R]
        wc_sb = const.tile([CW, D_MODEL], f16, tag="wc")
        ebias = const.tile([P, 1], f32, tag="ebias")
        nc.vector.memset(ebias[:], LN_RSQRT_D)

        # persistent per-chunk V tiles ([v_h0|1|v_h1|1]) with ones at 64/129
        va_t = []
        for c in range(NCH):
            va = const.tile([P, W2], f16, tag=f"va{c}", name=f"va{c}")
            ones_ap = va[:].rearrange("p (b c) -> p b c", c=W)[:, :, D]
            nc.vector.memset(ones_ap, 1.0)
            va_t.append(va)
        # persistent per-chunk block-diagonal fp16 prefix tiles (zeroed once)
        p16_t = []
        for c in range(1, NCH):
            p16 = const.tile([P, W2], f16, tag=f"p16_{c}", name=f"p16_{c}")
            nc.vector.memset(p16[:], 0.0)
            p16_t.append(p16)
        p16_t = [None] + p16_t  # index by chunk: pref16 for chunk c at [c]

        # ---- x input tiles, DMA'd st-major: (q,k,v) x st, 1MB per DMA ----
        xin = ctx.enter_context(tc.tile_pool(name="xin", bufs=1))
        x_t = {}
        for name in ("q", "k", "v"):
            x_t[name] = xin.tile([P, KT * S], f16, tag=f"x_{name}", name=f"x_{name}")
        for st in range(NST):
            sl = slice(st * ST, (st + 1) * ST)
            for name, srct in (("q", qT), ("k", kT), ("v", vT)):
                dst = x_t[name][:].rearrange("p (k s) -> p k s", k=KT)[:, :, sl]
                sr = srct[:, sl].rearrange("(k p) s -> p k s", p=P)
                if st == 0 and name in ("q", "k"):
                    # split into k-tile halves so the first projections can
                    # start as soon as half the s-tile has landed
                    h = KT // 2
                    nc.sync.dma_start(dst[:, 0:h, :], sr[:, 0:h, :])
                    nc.sync.dma_start(dst[:, h:KT, :], sr[:, h:KT, :])
                else:
                    nc.sync.dma_start(dst, sr)
            if st == 0:  # wc is first needed ~25us in; don't delay the x stream
                nc.sync.dma_start(wc_sb[:], wc[:, :])

        def xs(name, k, st):
            return x_t[name][:, k * S + st * ST : k * S + (st + 1) * ST]

        # ---- pools ----
        tmp_pool = ctx.enter_context(tc.tile_pool(name="tmp", bufs=3))
        # PSUM: 8 banks x 2KB/partition: bigp(3) + tpp(2) + sp(1) + atp(1) + op(1)
        big_psum = ctx.enter_context(tc.tile_pool(name="bigp", bufs=2, space="PSUM"))
        tp_psum = ctx.enter_context(tc.tile_pool(name="tpp", bufs=2, space="PSUM"))
        s_psum = ctx.enter_context(tc.tile_pool(name="sp", bufs=1, space="PSUM"))
        at_psum = ctx.enter_context(tc.tile_pool(name="atp", bufs=2, space="PSUM"))
        o_psum = ctx.enter_context(tc.tile_pool(name="op", bufs=1, space="PSUM"))
        qp_pool = ctx.enter_context(tc.tile_pool(name="qp", bufs=NST))
        kp_pool = ctx.enter_context(tc.tile_pool(name="kp", bufs=NST))
        vh_pool = ctx.enter_context(tc.tile_pool(name="vh", bufs=NST))
        sc_pool = ctx.enter_context(tc.tile_pool(name="sc", bufs=4))
        ks_pool = ctx.enter_context(tc.tile_pool(name="ks", bufs=8))
        prefF_pool = ctx.enter_context(tc.tile_pool(name="prF", bufs=2))
        ot_pool = ctx.enter_context(tc.tile_pool(name="ot", bufs=3))
        out_pool = ctx.enter_context(tc.tile_pool(name="outp", bufs=3))

        qp_t, kp_t, vh_t = [], [], []
        ks_t = [None] * NCH
        s_ps = [None] * NCH
        prefF = [None] * (NCH + 1)
        atm_t = [None] * NCH

        def emit_proj(name, st):
            pp = big_psum.tile([P, ST], f32, tag="big", name=f"prj_{name}{st}")
            for k in range(KT):
                nc.tensor.matmul(
                    pp[:], w_sb[("w" + name, k)][:], xs(name, k, st),
                    start=(k == 0), stop=(k == KT - 1)
                )
            return pp

        def emit_post_v(pp, st):
            vh = vh_pool.tile([P, ST], f16, tag="vh")
            nc.vector.tensor_scalar(vh[:], pp[:], b_sb["bv"][:], None, Alu.add)
            vh_t.append(vh)

        def emit_feat(name, pp, st):
            """q' = exp(qhg) * exp(-0.5*sum_d qhg^2 + ln(1/sqrt d)).
            Head-dim reduction via quadrant-packed ng matmuls."""
            tmp = tmp_pool.tile([P, ST], f16, tag=f"tmpl_{name}")
            nc.vector.tensor_scalar(tmp[:], pp[:], b_sb["b" + name][:], None, Alu.add)
            tmp2 = tmp_pool.tile([P, ST], f16, tag=f"tmps_{name}")
            nc.vector.tensor_tensor(tmp2[:], tmp[:], tmp[:], Alu.mult)
            fp = big_psum.tile([P, ST], f32, tag="big", name=f"phi_{name}{st}")
            nc.tensor.matmul(fp[0:D, :], ng_sb[0:D, :], tmp2[0:D, :],
                             start=True, stop=True)
            nc.tensor.matmul(fp[D:P, :], ng_sb[D:P, :], tmp2[D:P, :],
                             start=True, stop=True, tile_position=(D, D))
            e1 = tmp_pool.tile([P, ST], f16, tag=f"e1_{name}")
            nc.scalar.activation(e1[:], tmp[:], AF.Exp)
            e2 = tmp_pool.tile([P, ST], f16, tag=f"e2_{name}")
            nc.scalar.activation(e2[:], fp[:], AF.Exp, bias=ebias[:])
            dst_pool = qp_pool if name == "q" else kp_pool
            pt = dst_pool.tile([P, ST], f16, tag="qkp")
            nc.vector.tensor_tensor(pt[:], e1[:], e2[:], Alu.mult)
            (qp_t if name == "q" else kp_t).append(pt)

        def emit_tdma(st):
            """PE transposes of k' and v chunks to s-major (DMA-crossbar
            transposes cost ~1.3us queue dispatch each — too slow)."""
            for c in range(4 * st, 4 * st + 4):
                off = (c % 4) * C
                csl = slice(off, off + C)
                ktp = tp_psum.tile([P, P], f16, tag="tp", name=f"ktp{c}")
                nc.tensor.transpose(ktp[:], kp_t[st][:, csl], id_sb[:])
                ks = ks_pool.tile([P, P], f16, tag="ks", name=f"ks{c}")
                nc.vector.tensor_copy(ks[:], ktp[:])
                ks_t[c] = ks
                vtp = tp_psum.tile([P, P], f16, tag="tp", name=f"vtp{c}")
                nc.tensor.transpose(vtp[:], vh_t[st][:, csl], id_sb[:])
                va_dst = va_t[c][:].rearrange("p (b c) -> p b c", c=W)[:, :, 0:D]
                nc.scalar.activation(
                    va_dst, vtp[:].rearrange("p (b c) -> p b c", c=D), AF.Copy
                )

        def emit_state(c):
            """Per-chunk state (one matmul, both heads) + prefix step."""
            va = va_t[c]
            sp = s_psum.tile([P, W2], f32, tag="S", name=f"S{c}")
            nc.tensor.matmul(sp[:], ks_t[c][:], va[:], start=True, stop=True)
            s_ps[c] = sp
            # prefix: pref[c+1] = pref[c] + S_c (f32 vector); block-diag f16
            # copy on gpsimd for the next chunk's inter matmul
            pf = prefF_pool.tile([P, W2], f32, tag="prF")
            if c == 0:
                nc.vector.tensor_copy(pf[:], sp[:])
            else:
                nc.vector.tensor_tensor(pf[:], prefF[c][:], sp[:], Alu.add)
            prefF[c + 1] = pf
            if c + 1 < NCH:
                # O-inter reads only the diagonal blocks, so a full copy is fine
                nc.gpsimd.tensor_copy(p16_t[c + 1][:], pf[:])

        def emit_at(c):
            """Intra-chunk attention matrix + mask (both heads in one bank)."""
            st, off = c // 4, (c % 4) * C
            csl = slice(off, off + C)
            atm = []
            for h in range(HPC):
                atp = at_psum.tile([P, P], f32, tag="at", name=f"at{h}_{c}")
                nc.tensor.matmul(
                    atp[:], kp_t[st][h * D : (h + 1) * D, csl],
                    qp_t[st][h * D : (h + 1) * D, csl],
                    tile_position=(h * D, 0), start=True, stop=True,
                )
                am = sc_pool.tile([P, P], f16, tag=f"atm{h}", name=f"atm{h}_{c}")
                nc.vector.tensor_tensor(am[:], atp[:], mask_sb[:], Alu.mult)
                atm.append(am)
            atm_t[c] = atm

        o_ps = [None] * NCH
        osb_t = [None] * NCH

        def emit_o_mm(c):
            """O = intra + inter matmuls, then normalize on vector."""
            st, off = c // 4, (c % 4) * C
            csl = slice(off, off + C)
            va = va_t[c]
            op_t = o_psum.tile([P, W2], f32, tag="o", name=f"o_{c}")
            for h in range(HPC):
                nc.tensor.matmul(
                    op_t[:, h * W : (h + 1) * W], atm_t[c][h][:],
                    va[:, h * W : (h + 1) * W],
                    start=True, stop=(c == 0), skip_group_check=True,
                )
                if c > 0:
                    nc.tensor.matmul(
                        op_t[:, h * W : (h + 1) * W],
                        qp_t[st][h * D : (h + 1) * D, csl],
                        p16_t[c][h * D : (h + 1) * D, h * W : (h + 1) * W],
                        start=False, stop=True, skip_group_check=True,
                    )
            o_ps[c] = op_t
            rc = sc_pool.tile([P, HPC], f32, tag="rc")
            for h in range(HPC):
                nc.vector.reciprocal(rc[:, h : h + 1], op_t[:, h * W + D : h * W + D + 1])
            osb = sc_pool.tile([P, P], f16, tag="osb")
            for h in range(HPC):
                nc.vector.tensor_scalar(
                    osb[:, h * D : (h + 1) * D], op_t[:, h * W : h * W + D],
                    rc[:, h : h + 1], None, Alu.mult,
                )
            osb_t[c] = osb

        def emit_fin(c):
            """Transpose back, final projection, store."""
            otp = tp_psum.tile([P, P], f16, tag="tp", name=f"otp_{c}")
            nc.tensor.transpose(otp[:], osb_t[c][:], id_sb[:])
            ott = ot_pool.tile([P, P], f16, tag="ott")
            nc.vector.tensor_copy(ott[:], otp[:])
            ob = out_pool.tile([P, D_MODEL], f16, tag="ob")
            fps0 = big_psum.tile([P, ST], f32, tag="big", name=f"f0_{c}")
            nc.tensor.matmul(fps0[:], ott[:], wc_sb[:, 0:ST], start=True, stop=True)
            fps1 = big_psum.tile([P, ST], f32, tag="big", name=f"f1_{c}")
            nc.tensor.matmul(fps1[:], ott[:], wc_sb[:, ST:D_MODEL], start=True, stop=True)
            nc.scalar.activation(ob[:, 0:ST], fps0[:], AF.Copy)
            nc.scalar.activation(ob[:, ST:D_MODEL], fps1[:], AF.Copy)
            nc.sync.dma_start(out[c * C : (c + 1) * C, :], ob[:])

        import os
        STAGE = int(os.environ.get("KSTAGE", "9"))

        fin_done = [0]  # next chunk whose fin is pending

        def emit_chunks(st):
            # fin(c) is emitted one chunk late so the vector normalize of
            # chunk c overlaps chunk c+1's state/AT/O matmuls
            for c in range(4 * st, 4 * st + 4):
                if STAGE >= 2:
                    emit_state(c)
                if STAGE >= 3:
                    emit_at(c)
                if STAGE >= 4:
                    emit_o_mm(c)
                if STAGE >= 5:
                    while fin_done[0] < c:
                        emit_fin(fin_done[0])
                        fin_done[0] += 1
            if STAGE >= 5 and st == NST - 1:
                emit_fin(NCH - 1)

        # ---- interleaved emission: proj(st) | feat(st) | chunks(st-1) ----
        for st in range(NST):
            pq = emit_proj("q", st)
            pk = emit_proj("k", st)
            emit_feat("q", pq, st)
            emit_feat("k", pk, st)
            pv = emit_proj("v", st)
            emit_post_v(pv, st)
            emit_tdma(st)
            if st > 0:
                emit_chunks(st - 1)
        emit_chunks(NST - 1)

    nc.finalize()
    return nc


def _prep_inputs(v, k, q, wq_w, wq_b, wk_w, wk_b, wv_w, wv_b, wc_w, wc_b, wg):
    f16 = np.float16
    qT = np.ascontiguousarray(q[0].T).astype(f16)
    kT = np.ascontiguousarray(k[0].T).astype(f16)
    vT = np.ascontiguousarray(v[0].T).astype(f16)
    ident = np.eye(P, dtype=f16)
    mask = np.triu(np.ones((P, P), np.float32)).astype(f16)  # mask[j,i]=1 iff j<=i
    ng = np.full((P, R), -0.5, f16)
    aux = np.concatenate([ident, mask, ng], axis=1)  # (128, 320)

    # fold wg into q/k projections (wg orthogonal: ||x@wg|| == ||x||)
    wg32 = wg.astype(np.float32)
    def fold(w, b):
        wf = np.zeros((D_MODEL, D_MODEL), np.float32)
        bf = np.zeros(D_MODEL, np.float32)
        for h in range(N_HEADS):
            sl = slice(h * D, (h + 1) * D)
            wf[:, sl] = (w[:, sl].astype(np.float32) * NORM_D) @ wg32
            bf[sl] = (b[sl].astype(np.float32) * NORM_D) @ wg32
        return wf, bf
    wqg, bqg = fold(wq_w, wq_b)
    wkg, bkg = fold(wk_w, wk_b)

    def warr(w):  # [1024, 128] -> [128, 8*128] with [p, k*128+c] = w[k*128+p, c]
        return np.ascontiguousarray(
            w.reshape(KT, P, CW).transpose(1, 0, 2).reshape(P, KT * CW)
        ).astype(f16)

    in_maps = []
    for c in range(N_CORES):
        cs = slice(c * CW, (c + 1) * CW)
        bqkv = np.stack([
            bqg[cs].astype(np.float32),
            bkg[cs].astype(np.float32),
            wv_b[cs].astype(np.float32),
        ], axis=1)
        in_maps.append({
            "qT": qT, "kT": kT, "vT": vT,
            "wq": warr(wqg[:, cs]),
            "wk": warr(wkg[:, cs]),
            "wv": warr(wv_w[:, cs].astype(np.float32)),
            "bqkv": bqkv,
            "aux": aux,
            "wc": wc_w[cs, :].astype(f16),
        })
    return in_maps


def kernel(**inputs):
    from concourse.bass_utils import run_bass_kernel_spmd

    if "nc" not in _CACHE:
        _CACHE["nc"] = _build_bass()
    nc = _CACHE["nc"]
    in_maps = _prep_inputs(**inputs)
    res = run_bass_kernel_spmd(nc, in_maps, core_ids=list(range(N_CORES)))
    _CACHE["last_results"] = res
    acc = np.zeros((S, D_MODEL), np.float32)
    for c in range(N_CORES):
        acc += res.results[c]["out"].astype(np.float32)
    acc += inputs["wc_b"].astype(np.float32)[None, :]
    return acc[None, :, :]


if __name__ == "__main__":
    import reference

    inp = {k: np.asarray(v) for k, v in reference.setup_inputs().items()}
    got = kernel(**inp)
    print("kernel out", got.shape, got.dtype)


# revision 60
# speedup vs baseline: 1.0198x; 1.0198x over previous
"""Trainium2 Bass kernel for Performer-style (FAVOR+) causal linear attention.

Reference computation (per batch b=1, heads h=16, seq s=2048, d=64, r=64):
  qh = split_heads((q @ wq + bq) * d^-0.25)     kh likewise, vh = split_heads(v @ wv + bv)
  q' = (1/sqrt(d)) * exp(qh @ wg - 0.5*||qh||^2)   k' likewise
  attn[s] = (q'_s . sum_{j<=s} k'_j v_j^T) / (eps + q'_s . sum_{j<=s} k'_j)
  out = merge_heads(attn) @ wc + bc

Key simplifications:
  - wg is orthogonal (64x64 from QR), so ||qh||^2 == ||qh @ wg||^2. Folding
    wg into the projection weights (wqg = norm * wq @ blockdiag(wg)) means
    the kernel only computes qhg = q @ wqg and
    q' = exp(qhg) * exp(-0.5*sum_r qhg_r^2) / sqrt(d).
  - The causal scan is de-serialized: each chunk's state is an independent
    single matmul (both heads packed via the augmented-value layout);
    prefix states accumulate on the vector engine, with an fp16 copy on
    GpSimd feeding the inter-chunk matmuls.

Sharding: 2 heads per core (16 heads over 8 cores). Each core gets full
fp16 q/k/v (transposed) + its 128-column weight slices, computes its heads'
attention, projects through its 128-row slice of wc, and returns a
(2048, 1024) fp16 partial. The host sums the 8 partials and adds wc_b.
"""

import sys

if "/opt/trn_rl_repo" not in sys.path:
    sys.path.insert(0, "/opt/trn_rl_repo")

import math
from contextlib import ExitStack

import numpy as np

D_MODEL = 1024
N_HEADS = 16
D = 64  # head depth
R = 64  # kernel features (= D, wg orthogonal)
S = 2048
N_CORES = 8
HPC = N_HEADS // N_CORES  # heads per core = 2
CW = HPC * D  # per-core channel width = 128
P = 128
ST = 512  # projection s-tile width
NST = S // ST  # 4
C = 128  # scan chunk
NCH = S // C  # 16
KT = D_MODEL // P  # 8 contraction tiles
W = D + 1  # augmented value width (v | 1)
W2 = HPC * W  # 130
NORM_D = float(D ** (-0.25))
LN_RSQRT_D = float(-0.5 * math.log(D))  # exp(x + this) = exp(x)/sqrt(d)

_CACHE = {}


def _build_bass():
    import concourse.bass as bass
    import concourse.mybir as mybir
    import concourse.tile as tile
    from concourse.bacc import Bacc

    f16 = mybir.dt.float16
    f32 = mybir.dt.float32
    AF = mybir.ActivationFunctionType
    Alu = mybir.AluOpType

    nc = Bacc(trn_type="TRN2")

    qT = nc.dram_tensor("qT", [D_MODEL, S], f16, kind="ExternalInput")
    kT = nc.dram_tensor("kT", [D_MODEL, S], f16, kind="ExternalInput")
    vT = nc.dram_tensor("vT", [D_MODEL, S], f16, kind="ExternalInput")
    # weights host-prearranged to [128, k*cw] so the DMA is flat
    wq = nc.dram_tensor("wq", [P, KT * CW], f16, kind="ExternalInput")
    wk = nc.dram_tensor("wk", [P, KT * CW], f16, kind="ExternalInput")
    wv = nc.dram_tensor("wv", [P, KT * CW], f16, kind="ExternalInput")
    # aux: [ident(128) | mask(128) | ng(64)] packed along free dim
    aux = nc.dram_tensor("aux", [P, 2 * P + R], f16, kind="ExternalInput")
    bqkv = nc.dram_tensor("bqkv", [CW, 3], f32, kind="ExternalInput")
    wc = nc.dram_tensor("wc", [CW, D_MODEL], f16, kind="ExternalInput")
    out = nc.dram_tensor("out", [S, D_MODEL], f16, kind="ExternalOutput")

    with tile.TileContext(nc) as tc, ExitStack() as ctx:
        # ---- constant / weight tiles (sync queue, ahead of the x stream) ----
        const = ctx.enter_context(tc.tile_pool(name="const", bufs=1))
        w_sb = {}
        for name, drt in (("wq", wq), ("wk", wk), ("wv", wv)):
            t = const.tile([P, KT * CW], f16, tag=name, name=f"wt_{name}")
            nc.sync.dma_start(t[:], drt[:, :])
            for k in range(KT):
                w_sb[(name, k)] = t[:, k * CW : (k + 1) * CW]
        b_all = const.tile([CW, 3], f32, tag="ball")
        nc.sync.dma_start(b_all[:], bqkv[:, :])
        b_sb = {"bq": b_all[:, 0:1], "bk": b_all[:, 1:2], "bv": b_all[:, 2:3]}
        aux_sb = const.tile([P, 2 * P + R], f16, tag="aux")
        nc.sync.dma_start(aux_sb[:], aux[:, :])
        id_sb = aux_sb[:, 0:P]
        mask_sb = aux_sb[:, P : 2 * P]
        ng_sb = aux_sb[:, 2 * P : 2 * P + R]
        wc_sb = const.tile([CW, D_MODEL], f16, tag="wc")
        ebias = const.tile([P, 1], f32, tag="ebias")
        nc.vector.memset(ebias[:], LN_RSQRT_D)

        # persistent per-chunk V tiles ([v_h0|1|v_h1|1]) with ones at 64/129
        va_t = []
        for c in range(NCH):
            va = const.tile([P, W2], f16, tag=f"va{c}", name=f"va{c}")
            ones_ap = va[:].rearrange("p (b c) -> p b c", c=W)[:, :, D]
            nc.vector.memset(ones_ap, 1.0)
            va_t.append(va)
        # persistent per-chunk block-diagonal fp16 prefix tiles (zeroed once)
        p16_t = []
        for c in range(1, NCH):
            p16 = const.tile([P, W2], f16, tag=f"p16_{c}", name=f"p16_{c}")
            nc.vector.memset(p16[:], 0.0)
            p16_t.append(p16)
        p16_t = [None] + p16_t  # index by chunk: pref16 for chunk c at [c]

        # ---- x input tiles, DMA'd st-major: (q,k,v) x st, 1MB per DMA ----
        xin = ctx.enter_context(tc.tile_pool(name="xin", bufs=1))
        x_t = {}
        for name in ("q", "k", "v"):
            x_t[name] = xin.tile([P, KT * S], f16, tag=f"x_{name}", name=f"x_{name}")
        for st in range(NST):
            sl = slice(st * ST, (st + 1) * ST)
            for name, srct in (("q", qT), ("k", kT), ("v", vT)):
                dst = x_t[name][:].rearrange("p (k s) -> p k s", k=KT)[:, :, sl]
                sr = srct[:, sl].rearrange("(k p) s -> p k s", p=P)
                if st == 0 and name in ("q", "k"):
                    # split into k-tile halves so the first projections can
                    # start as soon as half the s-tile has landed
                    h = KT // 2
                    nc.sync.dma_start(dst[:, 0:h, :], sr[:, 0:h, :])
                    nc.sync.dma_start(dst[:, h:KT, :], sr[:, h:KT, :])
                else:
                    nc.sync.dma_start(dst, sr)
            if st == 0:  # wc is first needed ~25us in; don't delay the x stream
                nc.sync.dma_start(wc_sb[:], wc[:, :])

        def xs(name, k, st):
            return x_t[name][:, k * S + st * ST : k * S + (st + 1) * ST]

        # ---- pools ----
        tmp_pool = ctx.enter_context(tc.tile_pool(name="tmp", bufs=3))
        # PSUM: 8 banks x 2KB/partition: bigp(3) + tpp(2) + sp(1) + atp(1) + op(1)
        big_psum = ctx.enter_context(tc.tile_pool(name="bigp", bufs=2, space="PSUM"))
        tp_psum = ctx.enter_context(tc.tile_pool(name="tpp", bufs=2, space="PSUM"))
        s_psum = ctx.enter_context(tc.tile_pool(name="sp", bufs=1, space="PSUM"))
        at_psum = ctx.enter_context(tc.tile_pool(name="atp", bufs=2, space="PSUM"))
        o_psum = ctx.enter_context(tc.tile_pool(name="op", bufs=1, space="PSUM"))
        qp_pool = ctx.enter_context(tc.tile_pool(name="qp", bufs=NST))
        kp_pool = ctx.enter_context(tc.tile_pool(name="kp", bufs=NST))
        vh_pool = ctx.enter_context(tc.tile_pool(name="vh", bufs=NST))
        sc_pool = ctx.enter_context(tc.tile_pool(name="sc", bufs=4))
        ks_pool = ctx.enter_context(tc.tile_pool(name="ks", bufs=8))
        prefF_pool = ctx.enter_context(tc.tile_pool(name="prF", bufs=2))
        ot_pool = ctx.enter_context(tc.tile_pool(name="ot", bufs=3))
        out_pool = ctx.enter_context(tc.tile_pool(name="outp", bufs=3))

        qp_t, kp_t, vh_t = [], [], []
        ks_t = [None] * NCH
        s_ps = [None] * NCH
        prefF = [None] * (NCH + 1)
        atm_t = [None] * NCH

        def emit_proj(name, st):
            pp = big_psum.tile([P, ST], f32, tag="big", name=f"prj_{name}{st}")
            for k in range(KT):
                nc.tensor.matmul(
                    pp[:], w_sb[("w" + name, k)][:], xs(name, k, st),
                    start=(k == 0), stop=(k == KT - 1)
                )
            return pp

        def emit_post_v(pp, st):
            vh = vh_pool.tile([P, ST], f16, tag="vh")
            nc.vector.tensor_scalar(vh[:], pp[:], b_sb["bv"][:], None, Alu.add)
            vh_t.append(vh)

        def emit_feat(name, pp, st):
            """q' = exp(qhg) * exp(-0.5*sum_d qhg^2 + ln(1/sqrt d)).
            Head-dim reduction via quadrant-packed ng matmuls."""
            tmp = tmp_pool.tile([P, ST], f16, tag=f"tmpl_{name}")
            nc.vector.tensor_scalar(tmp[:], pp[:], b_sb["b" + name][:], None, Alu.add)
            tmp2 = tmp_pool.tile([P, ST], f16, tag=f"tmps_{name}")
            nc.vector.tensor_tensor(tmp2[:], tmp[:], tmp[:], Alu.mult)
            fp = big_psum.tile([P, ST], f32, tag="big", name=f"phi_{name}{st}")
            nc.tensor.matmul(fp[0:D, :], ng_sb[0:D, :], tmp2[0:D, :],
                             start=True, stop=True)
            nc.tensor.matmul(fp[D:P, :], ng_sb[D:P, :], tmp2[D:P, :],
                             start=True, stop=True, tile_position=(D, D))
            e1 = tmp_pool.tile([P, ST], f16, tag=f"e1_{name}")
            nc.scalar.activation(e1[:], tmp[:], AF.Exp)
            e2 = tmp_pool.tile([P, ST], f16, tag=f"e2_{name}")
            nc.scalar.activation(e2[:], fp[:], AF.Exp, bias=ebias[:])
            dst_pool = qp_pool if name == "q" else kp_pool
            pt = dst_pool.tile([P, ST], f16, tag="qkp")
            nc.vector.tensor_tensor(pt[:], e1[:], e2[:], Alu.mult)
            (qp_t if name == "q" else kp_t).append(pt)

        def emit_tdma(st):
            """PE transposes of k' and v chunks to s-major (DMA-crossbar
            transposes cost ~1.3us queue dispatch each — too slow)."""
            for c in range(4 * st, 4 * st + 4):
                off = (c % 4) * C
                csl = slice(off, off + C)
                ktp = tp_psum.tile([P, P], f16, tag="tp", name=f"ktp{c}")
                nc.tensor.transpose(ktp[:], kp_t[st][:, csl], id_sb[:])
                ks = ks_pool.tile([P, P], f16, tag="ks", name=f"ks{c}")
                nc.vector.tensor_copy(ks[:], ktp[:])
                ks_t[c] = ks
                vtp = tp_psum.tile([P, P], f16, tag="tp", name=f"vtp{c}")
                nc.tensor.transpose(vtp[:], vh_t[st][:, csl], id_sb[:])
                va_dst = va_t[c][:].rearrange("p (b c) -> p b c", c=W)[:, :, 0:D]
                nc.scalar.activation(
                    va_dst, vtp[:].rearrange("p (b c) -> p b c", c=D), AF.Copy
                )

        def emit_state(c):
            """Per-chunk state (one matmul, both heads) + prefix step."""
            va = va_t[c]
            sp = s_psum.tile([P, W2], f32, tag="S", name=f"S{c}")
            nc.tensor.matmul(sp[:], ks_t[c][:], va[:], start=True, stop=True)
            s_ps[c] = sp
            # prefix: pref[c+1] = pref[c] + S_c (f32 vector); block-diag f16
            # copy on gpsimd for the next chunk's inter matmul
            pf = prefF_pool.tile([P, W2], f32, tag="prF")
            if c == 0:
                nc.vector.tensor_copy(pf[:], sp[:])
            else:
                nc.vector.tensor_tensor(pf[:], prefF[c][:], sp[:], Alu.add)
            prefF[c + 1] = pf
            if c + 1 < NCH:
                # O-inter reads only the diagonal blocks, so a full copy is fine
                nc.gpsimd.tensor_copy(p16_t[c + 1][:], pf[:])

        def emit_at(c):
            """Intra-chunk attention matrix + mask (both heads in one bank)."""
            st, off = c // 4, (c % 4) * C
            csl = slice(off, off + C)
            atm = []
            for h in range(HPC):
                atp = at_psum.tile([P, P], f32, tag="at", name=f"at{h}_{c}")
                nc.tensor.matmul(
                    atp[:], kp_t[st][h * D : (h + 1) * D, csl],
                    qp_t[st][h * D : (h + 1) * D, csl],
                    tile_position=(h * D, 0), start=True, stop=True,
                )
                am = sc_pool.tile([P, P], f16, tag=f"atm{h}", name=f"atm{h}_{c}")
                nc.vector.tensor_tensor(am[:], atp[:], mask_sb[:], Alu.mult)
                atm.append(am)
            atm_t[c] = atm

        o_ps = [None] * NCH
        osb_t = [None] * NCH

        def emit_o_mm(c):
            """O = intra + inter matmuls, then normalize on vector."""
            st, off = c // 4, (c % 4) * C
            csl = slice(off, off + C)
            va = va_t[c]
            op_t = o_psum.tile([P, W2], f32, tag="o", name=f"o_{c}")
            for h in range(HPC):
                nc.tensor.matmul(
                    op_t[:, h * W : (h + 1) * W], atm_t[c][h][:],
                    va[:, h * W : (h + 1) * W],
                    start=True, stop=(c == 0), skip_group_check=True,
                )
                if c > 0:
                    nc.tensor.matmul(
                        op_t[:, h * W : (h + 1) * W],
                        qp_t[st][h * D : (h + 1) * D, csl],
                        p16_t[c][h * D : (h + 1) * D, h * W : (h + 1) * W],
                        start=False, stop=True, skip_group_check=True,
                    )
            o_ps[c] = op_t
            rc = sc_pool.tile([P, HPC], f32, tag="rc")
            for h in range(HPC):
                nc.vector.reciprocal(rc[:, h : h + 1], op_t[:, h * W + D : h * W + D + 1])
            osb = sc_pool.tile([P, P], f16, tag="osb")
            for h in range(HPC):
                nc.vector.tensor_scalar(
                    osb[:, h * D : (h + 1) * D], op_t[:, h * W : h * W + D],
                    rc[:, h : h + 1], None, Alu.mult,
                )
            osb_t[c] = osb

        def emit_fin(c):
            """Transpose back, final projection, store."""
            otp = tp_psum.tile([P, P], f16, tag="tp", name=f"otp_{c}")
            nc.tensor.transpose(otp[:], osb_t[c][:], id_sb[:])
            ott = ot_pool.tile([P, P], f16, tag="ott")
            nc.vector.tensor_copy(ott[:], otp[:])
            ob = out_pool.tile([P, D_MODEL], f16, tag="ob")
            fps0 = big_psum.tile([P, ST], f32, tag="big", name=f"f0_{c}")
            nc.tensor.matmul(fps0[:], ott[:], wc_sb[:, 0:ST], start=True, stop=True)
            fps1 = big_psum.tile([P, ST], f32, tag="big", name=f"f1_{c}")
            nc.tensor.matmul(fps1[:], ott[:], wc_sb[:, ST:D_MODEL], start=True, stop=True)
            nc.scalar.activation(ob[:, 0:ST], fps0[:], AF.Copy)
            nc.vector.tensor_copy(ob[:, ST:D_MODEL], fps1[:])
            nc.scalar.dma_start(out[c * C : (c + 1) * C, :], ob[:])

        import os
        STAGE = int(os.environ.get("KSTAGE", "9"))

        fin_done = [0]  # next chunk whose fin is pending

        def emit_chunks(st):
            # fin(c) is emitted one chunk late so the vector normalize of
            # chunk c overlaps chunk c+1's state/AT/O matmuls
            for c in range(4 * st, 4 * st + 4):
                if STAGE >= 2:
                    emit_state(c)
                if STAGE >= 3:
                    emit_at(c)
                if STAGE >= 4:
                    emit_o_mm(c)
                if STAGE >= 5:
                    while fin_done[0] < c:
                        emit_fin(fin_done[0])
                        fin_done[0] += 1
            if STAGE >= 5 and st == NST - 1:
                emit_fin(NCH - 1)

        # ---- interleaved emission: proj(st) | feat(st) | chunks(st-1) ----
        for st in range(NST):
            pq = emit_proj("q", st)
            pk = emit_proj("k", st)
            emit_feat("q", pq, st)
            emit_feat("k", pk, st)
            pv = emit_proj("v", st)
            emit_post_v(pv, st)
            emit_tdma(st)
            if st > 0:
                emit_chunks(st - 1)
        emit_chunks(NST - 1)

    nc.finalize()
    return nc


def _prep_inputs(v, k, q, wq_w, wq_b, wk_w, wk_b, wv_w, wv_b, wc_w, wc_b, wg):
    f16 = np.float16
    qT = np.ascontiguousarray(q[0].T).astype(f16)
    kT = np.ascontiguousarray(k[0].T).astype(f16)
    vT = np.ascontiguousarray(v[0].T).astype(f16)
    ident = np.eye(P, dtype=f16)
    mask = np.triu(np.ones((P, P), np.float32)).astype(f16)  # mask[j,i]=1 iff j<=i
    ng = np.full((P, R), -0.5, f16)
    aux = np.concatenate([ident, mask, ng], axis=1)  # (128, 320)

    # fold wg into q/k projections (wg orthogonal: ||x@wg|| == ||x||)
    wg32 = wg.astype(np.float32)
    def fold(w, b):
        wf = np.zeros((D_MODEL, D_MODEL), np.float32)
        bf = np.zeros(D_MODEL, np.float32)
        for h in range(N_HEADS):
            sl = slice(h * D, (h + 1) * D)
            wf[:, sl] = (w[:, sl].astype(np.float32) * NORM_D) @ wg32
            bf[sl] = (b[sl].astype(np.float32) * NORM_D) @ wg32
        return wf, bf
    wqg, bqg = fold(wq_w, wq_b)
    wkg, bkg = fold(wk_w, wk_b)

    def warr(w):  # [1024, 128] -> [128, 8*128] with [p, k*128+c] = w[k*128+p, c]
        return np.ascontiguousarray(
            w.reshape(KT, P, CW).transpose(1, 0, 2).reshape(P, KT * CW)
        ).astype(f16)

    in_maps = []
    for c in range(N_CORES):
        cs = slice(c * CW, (c + 1) * CW)
        bqkv = np.stack([
            bqg[cs].astype(np.float32),
            bkg[cs].astype(np.float32),
            wv_b[cs].astype(np.float32),
        ], axis=1)
        in_maps.append({
            "qT": qT, "kT": kT, "vT": vT,
            "wq": warr(wqg[:, cs]),
            "wk": warr(wkg[:, cs]),
            "wv": warr(wv_w[:, cs].astype(np.float32)),
            "bqkv": bqkv,
            "aux": aux,
            "wc": wc_w[cs, :].astype(f16),
        })
    return in_maps


def kernel(**inputs):
    from concourse.bass_utils import run_bass_kernel_spmd

    if "nc" not in _CACHE:
        _CACHE["nc"] = _build_bass()
    nc = _CACHE["nc"]
    in_maps = _prep_inputs(**inputs)
    res = run_bass_kernel_spmd(nc, in_maps, core_ids=list(range(N_CORES)))
    _CACHE["last_results"] = res
    acc = np.zeros((S, D_MODEL), np.float32)
    for c in range(N_CORES):
        acc += res.results[c]["out"].astype(np.float32)
    acc += inputs["wc_b"].astype(np.float32)[None, :]
    return acc[None, :, :]


if __name__ == "__main__":
    import reference

    inp = {k: np.asarray(v) for k, v in reference.setup_inputs().items()}
    got = kernel(**inp)
    print("kernel out", got.shape, got.dtype)


# revision 61
# speedup vs baseline: 1.0280x; 1.0080x over previous
"""Trainium2 Bass kernel for Performer-style (FAVOR+) causal linear attention.

Reference computation (per batch b=1, heads h=16, seq s=2048, d=64, r=64):
  qh = split_heads((q @ wq + bq) * d^-0.25)     kh likewise, vh = split_heads(v @ wv + bv)
  q' = (1/sqrt(d)) * exp(qh @ wg - 0.5*||qh||^2)   k' likewise
  attn[s] = (q'_s . sum_{j<=s} k'_j v_j^T) / (eps + q'_s . sum_{j<=s} k'_j)
  out = merge_heads(attn) @ wc + bc

Key simplifications:
  - wg is orthogonal (64x64 from QR), so ||qh||^2 == ||qh @ wg||^2. Folding
    wg into the projection weights (wqg = norm * wq @ blockdiag(wg)) means
    the kernel only computes qhg = q @ wqg and
    q' = exp(qhg) * exp(-0.5*sum_r qhg_r^2) / sqrt(d).
  - The causal scan is de-serialized: each chunk's state is an independent
    single matmul (both heads packed via the augmented-value layout);
    prefix states accumulate on the vector engine, with an fp16 copy on
    GpSimd feeding the inter-chunk matmuls.

Sharding: 2 heads per core (16 heads over 8 cores). Each core gets full
fp16 q/k/v (transposed) + its 128-column weight slices, computes its heads'
attention, projects through its 128-row slice of wc, and returns a
(2048, 1024) fp16 partial. The host sums the 8 partials and adds wc_b.
"""

import sys

if "/opt/trn_rl_repo" not in sys.path:
    sys.path.insert(0, "/opt/trn_rl_repo")

import math
from contextlib import ExitStack

import numpy as np

D_MODEL = 1024
N_HEADS = 16
D = 64  # head depth
R = 64  # kernel features (= D, wg orthogonal)
S = 2048
N_CORES = 8
HPC = N_HEADS // N_CORES  # heads per core = 2
CW = HPC * D  # per-core channel width = 128
P = 128
ST = 512  # projection s-tile width
NST = S // ST  # 4
C = 128  # scan chunk
NCH = S // C  # 16
KT = D_MODEL // P  # 8 contraction tiles
W = D + 1  # augmented value width (v | 1)
W2 = HPC * W  # 130
NORM_D = float(D ** (-0.25))
LN_RSQRT_D = float(-0.5 * math.log(D))  # exp(x + this) = exp(x)/sqrt(d)

_CACHE = {}


def _build_bass():
    import concourse.bass as bass
    import concourse.mybir as mybir
    import concourse.tile as tile
    from concourse.bacc import Bacc

    f16 = mybir.dt.float16
    f32 = mybir.dt.float32
    AF = mybir.ActivationFunctionType
    Alu = mybir.AluOpType

    nc = Bacc(trn_type="TRN2")

    qT = nc.dram_tensor("qT", [D_MODEL, S], f16, kind="ExternalInput")
    kT = nc.dram_tensor("kT", [D_MODEL, S], f16, kind="ExternalInput")
    vT = nc.dram_tensor("vT", [D_MODEL, S], f16, kind="ExternalInput")
    # weights host-prearranged to [128, k*cw] so the DMA is flat
    wq = nc.dram_tensor("wq", [P, KT * CW], f16, kind="ExternalInput")
    wk = nc.dram_tensor("wk", [P, KT * CW], f16, kind="ExternalInput")
    wv = nc.dram_tensor("wv", [P, KT * CW], f16, kind="ExternalInput")
    # aux: [ident(128) | mask(128) | ng(64)] packed along free dim
    aux = nc.dram_tensor("aux", [P, 2 * P + R], f16, kind="ExternalInput")
    bqkv = nc.dram_tensor("bqkv", [CW, 3], f32, kind="ExternalInput")
    wc = nc.dram_tensor("wc", [CW, D_MODEL], f16, kind="ExternalInput")
    out = nc.dram_tensor("out", [S, D_MODEL], f16, kind="ExternalOutput")

    with tile.TileContext(nc) as tc, ExitStack() as ctx:
        # ---- constant / weight tiles (sync queue, ahead of the x stream) ----
        const = ctx.enter_context(tc.tile_pool(name="const", bufs=1))
        w_sb = {}
        for name, drt in (("wq", wq), ("wk", wk), ("wv", wv)):
            t = const.tile([P, KT * CW], f16, tag=name, name=f"wt_{name}")
            nc.sync.dma_start(t[:], drt[:, :])
            for k in range(KT):
                w_sb[(name, k)] = t[:, k * CW : (k + 1) * CW]
        b_all = const.tile([CW, 3], f32, tag="ball")
        nc.sync.dma_start(b_all[:], bqkv[:, :])
        b_sb = {"bq": b_all[:, 0:1], "bk": b_all[:, 1:2], "bv": b_all[:, 2:3]}
        aux_sb = const.tile([P, 2 * P + R], f16, tag="aux")
        nc.sync.dma_start(aux_sb[:], aux[:, :])
        id_sb = aux_sb[:, 0:P]
        mask_sb = aux_sb[:, P : 2 * P]
        ng_sb = aux_sb[:, 2 * P : 2 * P + R]
        wc_sb = const.tile([CW, D_MODEL], f16, tag="wc")
        ebias = const.tile([P, 1], f32, tag="ebias")
        nc.vector.memset(ebias[:], LN_RSQRT_D)

        # persistent per-chunk V tiles ([v_h0|1|v_h1|1]) with ones at 64/129
        va_t = []
        for c in range(NCH):
            va = const.tile([P, W2], f16, tag=f"va{c}", name=f"va{c}")
            ones_ap = va[:].rearrange("p (b c) -> p b c", c=W)[:, :, D]
            nc.vector.memset(ones_ap, 1.0)
            va_t.append(va)
        # persistent per-chunk block-diagonal fp16 prefix tiles (zeroed once)
        p16_t = []
        for c in range(1, NCH):
            p16 = const.tile([P, W2], f16, tag=f"p16_{c}", name=f"p16_{c}")
            nc.vector.memset(p16[:], 0.0)
            p16_t.append(p16)
        p16_t = [None] + p16_t  # index by chunk: pref16 for chunk c at [c]

        # ---- x input tiles, DMA'd st-major: (q,k,v) x st, 1MB per DMA ----
        xin = ctx.enter_context(tc.tile_pool(name="xin", bufs=1))
        x_t = {}
        for name in ("q", "k", "v"):
            x_t[name] = xin.tile([P, KT * S], f16, tag=f"x_{name}", name=f"x_{name}")
        for st in range(NST):
            sl = slice(st * ST, (st + 1) * ST)
            for name, srct in (("q", qT), ("k", kT), ("v", vT)):
                dst = x_t[name][:].rearrange("p (k s) -> p k s", k=KT)[:, :, sl]
                sr = srct[:, sl].rearrange("(k p) s -> p k s", p=P)
                if st == 0 and name in ("q", "k"):
                    # split into k-tile halves so the first projections can
                    # start as soon as half the s-tile has landed
                    h = KT // 2
                    nc.sync.dma_start(dst[:, 0:h, :], sr[:, 0:h, :])
                    nc.sync.dma_start(dst[:, h:KT, :], sr[:, h:KT, :])
                else:
                    nc.sync.dma_start(dst, sr)
            if st == 0:  # wc is first needed ~25us in; don't delay the x stream
                nc.sync.dma_start(wc_sb[:], wc[:, :])

        def xs(name, k, st):
            return x_t[name][:, k * S + st * ST : k * S + (st + 1) * ST]

        # ---- pools ----
        tmp_pool = ctx.enter_context(tc.tile_pool(name="tmp", bufs=3))
        # PSUM: 8 banks x 2KB/partition: bigp(3) + tpp(2) + sp(1) + atp(1) + op(1)
        big_psum = ctx.enter_context(tc.tile_pool(name="bigp", bufs=2, space="PSUM"))
        tp_psum = ctx.enter_context(tc.tile_pool(name="tpp", bufs=2, space="PSUM"))
        s_psum = ctx.enter_context(tc.tile_pool(name="sp", bufs=1, space="PSUM"))
        at_psum = ctx.enter_context(tc.tile_pool(name="atp", bufs=2, space="PSUM"))
        o_psum = ctx.enter_context(tc.tile_pool(name="op", bufs=1, space="PSUM"))
        qp_pool = ctx.enter_context(tc.tile_pool(name="qp", bufs=NST))
        kp_pool = ctx.enter_context(tc.tile_pool(name="kp", bufs=NST))
        vh_pool = ctx.enter_context(tc.tile_pool(name="vh", bufs=NST))
        sc_pool = ctx.enter_context(tc.tile_pool(name="sc", bufs=4))
        ks_pool = ctx.enter_context(tc.tile_pool(name="ks", bufs=8))
        prefF_pool = ctx.enter_context(tc.tile_pool(name="prF", bufs=2))
        ot_pool = ctx.enter_context(tc.tile_pool(name="ot", bufs=3))
        out_pool = ctx.enter_context(tc.tile_pool(name="outp", bufs=3))

        qp_t, kp_t, vh_t = [], [], []
        ks_t = [None] * NCH
        s_ps = [None] * NCH
        prefF = [None] * (NCH + 1)
        atm_t = [None] * NCH

        def emit_proj(name, st):
            pp = big_psum.tile([P, ST], f32, tag="big", name=f"prj_{name}{st}")
            for k in range(KT):
                nc.tensor.matmul(
                    pp[:], w_sb[("w" + name, k)][:], xs(name, k, st),
                    start=(k == 0), stop=(k == KT - 1)
                )
            return pp

        def emit_post_v(pp, st):
            vh = vh_pool.tile([P, ST], f16, tag="vh")
            nc.vector.tensor_scalar(vh[:], pp[:], b_sb["bv"][:], None, Alu.add)
            vh_t.append(vh)

        def emit_feat(name, pp, st):
            """q' = exp(qhg) * exp(-0.5*sum_d qhg^2 + ln(1/sqrt d)).
            Head-dim reduction via quadrant-packed ng matmuls."""
            tmp = tmp_pool.tile([P, ST], f16, tag=f"tmpl_{name}")
            nc.vector.tensor_scalar(tmp[:], pp[:], b_sb["b" + name][:], None, Alu.add)
            tmp2 = tmp_pool.tile([P, ST], f16, tag=f"tmps_{name}")
            nc.vector.tensor_tensor(tmp2[:], tmp[:], tmp[:], Alu.mult)
            fp = big_psum.tile([P, ST], f32, tag="big", name=f"phi_{name}{st}")
            nc.tensor.matmul(fp[0:D, :], ng_sb[0:D, :], tmp2[0:D, :],
                             start=True, stop=True)
            nc.tensor.matmul(fp[D:P, :], ng_sb[D:P, :], tmp2[D:P, :],
                             start=True, stop=True, tile_position=(D, D))
            e1 = tmp_pool.tile([P, ST], f16, tag=f"e1_{name}")
            nc.scalar.activation(e1[:], tmp[:], AF.Exp)
            e2 = tmp_pool.tile([P, ST], f16, tag=f"e2_{name}")
            nc.scalar.activation(e2[:], fp[:], AF.Exp, bias=ebias[:])
            dst_pool = qp_pool if name == "q" else kp_pool
            pt = dst_pool.tile([P, ST], f16, tag="qkp")
            nc.vector.tensor_tensor(pt[:], e1[:], e2[:], Alu.mult)
            (qp_t if name == "q" else kp_t).append(pt)

        def emit_tdma(st):
            """PE transposes of k' and v chunks to s-major (DMA-crossbar
            transposes cost ~1.3us queue dispatch each — too slow)."""
            for c in range(4 * st, 4 * st + 4):
                off = (c % 4) * C
                csl = slice(off, off + C)
                ktp = tp_psum.tile([P, P], f16, tag="tp", name=f"ktp{c}")
                nc.tensor.transpose(ktp[:], kp_t[st][:, csl], id_sb[:])
                ks = ks_pool.tile([P, P], f16, tag="ks", name=f"ks{c}")
                nc.vector.tensor_copy(ks[:], ktp[:])
                ks_t[c] = ks
                vtp = tp_psum.tile([P, P], f16, tag="tp", name=f"vtp{c}")
                nc.tensor.transpose(vtp[:], vh_t[st][:, csl], id_sb[:])
                va_dst = va_t[c][:].rearrange("p (b c) -> p b c", c=W)[:, :, 0:D]
                nc.scalar.activation(
                    va_dst, vtp[:].rearrange("p (b c) -> p b c", c=D), AF.Copy
                )

        def emit_state(c):
            """Per-chunk state (one matmul, both heads) + prefix step."""
            va = va_t[c]
            sp = s_psum.tile([P, W2], f32, tag="S", name=f"S{c}")
            nc.tensor.matmul(sp[:], ks_t[c][:], va[:], start=True, stop=True)
            s_ps[c] = sp
            # prefix: pref[c+1] = pref[c] + S_c (f32 vector); block-diag f16
            # copy on gpsimd for the next chunk's inter matmul
            pf = prefF_pool.tile([P, W2], f32, tag="prF")
            if c == 0:
                nc.vector.tensor_copy(pf[:], sp[:])
            else:
                nc.vector.tensor_tensor(pf[:], prefF[c][:], sp[:], Alu.add)
            prefF[c + 1] = pf
            if c + 1 < NCH:
                # O-inter reads only the diagonal blocks, so a full copy is fine
                nc.gpsimd.tensor_copy(p16_t[c + 1][:], pf[:])

        def emit_at(c):
            """Intra-chunk attention matrix + mask (both heads in one bank)."""
            st, off = c // 4, (c % 4) * C
            csl = slice(off, off + C)
            atm = []
            for h in range(HPC):
                atp = at_psum.tile([P, P], f32, tag="at", name=f"at{h}_{c}")
                nc.tensor.matmul(
                    atp[:], kp_t[st][h * D : (h + 1) * D, csl],
                    qp_t[st][h * D : (h + 1) * D, csl],
                    tile_position=(h * D, 0), start=True, stop=True,
                )
                am = sc_pool.tile([P, P], f16, tag=f"atm{h}", name=f"atm{h}_{c}")
                nc.vector.tensor_tensor(am[:], atp[:], mask_sb[:], Alu.mult)
                atm.append(am)
            atm_t[c] = atm

        o_ps = [None] * NCH
        osb_t = [None] * NCH

        def emit_o_mm(c):
            """O = intra + inter matmuls, then normalize on vector."""
            st, off = c // 4, (c % 4) * C
            csl = slice(off, off + C)
            va = va_t[c]
            op_t = o_psum.tile([P, W2], f32, tag="o", name=f"o_{c}")
            for h in range(HPC):
                nc.tensor.matmul(
                    op_t[:, h * W : (h + 1) * W], atm_t[c][h][:],
                    va[:, h * W : (h + 1) * W],
                    start=True, stop=(c == 0), skip_group_check=True,
                )
                if c > 0:
                    nc.tensor.matmul(
                        op_t[:, h * W : (h + 1) * W],
                        qp_t[st][h * D : (h + 1) * D, csl],
                        p16_t[c][h * D : (h + 1) * D, h * W : (h + 1) * W],
                        start=False, stop=True, skip_group_check=True,
                    )
            o_ps[c] = op_t
            rc = sc_pool.tile([P, HPC], f32, tag="rc")
            for h in range(HPC):
                nc.vector.reciprocal(rc[:, h : h + 1], op_t[:, h * W + D : h * W + D + 1])
            osb = sc_pool.tile([P, P], f16, tag="osb")
            for h in range(HPC):
                nc.vector.tensor_scalar(
                    osb[:, h * D : (h + 1) * D], op_t[:, h * W : h * W + D],
                    rc[:, h : h + 1], None, Alu.mult,
                )
            osb_t[c] = osb

        def emit_fin(c):
            """Transpose back, final projection, store."""
            otp = tp_psum.tile([P, P], f16, tag="tp", name=f"otp_{c}")
            nc.tensor.transpose(otp[:], osb_t[c][:], id_sb[:])
            ott = ot_pool.tile([P, P], f16, tag="ott")
            nc.vector.tensor_copy(ott[:], otp[:])
            ob = out_pool.tile([P, D_MODEL], f16, tag="ob")
            fps0 = big_psum.tile([P, ST], f32, tag="big", name=f"f0_{c}")
            nc.tensor.matmul(fps0[:], ott[:], wc_sb[:, 0:ST], start=True, stop=True)
            fps1 = big_psum.tile([P, ST], f32, tag="big", name=f"f1_{c}")
            nc.tensor.matmul(fps1[:], ott[:], wc_sb[:, ST:D_MODEL], start=True, stop=True)
            nc.scalar.activation(ob[:, 0:ST], fps0[:], AF.Copy)
            nc.vector.tensor_copy(ob[:, ST:D_MODEL], fps1[:])
            nc.scalar.dma_start(out[c * C : (c + 1) * C, :], ob[:])

        import os
        STAGE = int(os.environ.get("KSTAGE", "9"))

        fin_done = [0]  # next chunk whose fin is pending

        def emit_chunks(st):
            # fin(c-1) is emitted between chunk c's AT and O matmuls: the
            # tensor engine streams otp+finals of the previous chunk while
            # the vector engine masks this chunk's AT.
            for c in range(4 * st, 4 * st + 4):
                if STAGE >= 2:
                    emit_state(c)
                if STAGE >= 3:
                    emit_at(c)
                if STAGE >= 5:
                    while fin_done[0] < c:
                        emit_fin(fin_done[0])
                        fin_done[0] += 1
                if STAGE >= 4:
                    emit_o_mm(c)
            if STAGE >= 5 and st == NST - 1:
                emit_fin(NCH - 1)

        # ---- interleaved emission: proj(st) | feat(st) | chunks(st-1) ----
        for st in range(NST):
            pq = emit_proj("q", st)
            pk = emit_proj("k", st)
            emit_feat("q", pq, st)
            emit_feat("k", pk, st)
            pv = emit_proj("v", st)
            emit_post_v(pv, st)
            emit_tdma(st)
            if st > 0:
                emit_chunks(st - 1)
        emit_chunks(NST - 1)

    nc.finalize()
    return nc


def _prep_inputs(v, k, q, wq_w, wq_b, wk_w, wk_b, wv_w, wv_b, wc_w, wc_b, wg):
    f16 = np.float16
    qT = np.ascontiguousarray(q[0].T).astype(f16)
    kT = np.ascontiguousarray(k[0].T).astype(f16)
    vT = np.ascontiguousarray(v[0].T).astype(f16)
    ident = np.eye(P, dtype=f16)
    mask = np.triu(np.ones((P, P), np.float32)).astype(f16)  # mask[j,i]=1 iff j<=i
    ng = np.full((P, R), -0.5, f16)
    aux = np.concatenate([ident, mask, ng], axis=1)  # (128, 320)

    # fold wg into q/k projections (wg orthogonal: ||x@wg|| == ||x||)
    wg32 = wg.astype(np.float32)
    def fold(w, b):
        wf = np.zeros((D_MODEL, D_MODEL), np.float32)
        bf = np.zeros(D_MODEL, np.float32)
        for h in range(N_HEADS):
            sl = slice(h * D, (h + 1) * D)
            wf[:, sl] = (w[:, sl].astype(np.float32) * NORM_D) @ wg32
            bf[sl] = (b[sl].astype(np.float32) * NORM_D) @ wg32
        return wf, bf
    wqg, bqg = fold(wq_w, wq_b)
    wkg, bkg = fold(wk_w, wk_b)

    def warr(w):  # [1024, 128] -> [128, 8*128] with [p, k*128+c] = w[k*128+p, c]
        return np.ascontiguousarray(
            w.reshape(KT, P, CW).transpose(1, 0, 2).reshape(P, KT * CW)
        ).astype(f16)

    in_maps = []
    for c in range(N_CORES):
        cs = slice(c * CW, (c + 1) * CW)
        bqkv = np.stack([
            bqg[cs].astype(np.float32),
            bkg[cs].astype(np.float32),
            wv_b[cs].astype(np.float32),
        ], axis=1)
        in_maps.append({
            "qT": qT, "kT": kT, "vT": vT,
            "wq": warr(wqg[:, cs]),
            "wk": warr(wkg[:, cs]),
            "wv": warr(wv_w[:, cs].astype(np.float32)),
            "bqkv": bqkv,
            "aux": aux,
            "wc": wc_w[cs, :].astype(f16),
        })
    return in_maps


def kernel(**inputs):
    from concourse.bass_utils import run_bass_kernel_spmd

    if "nc" not in _CACHE:
        _CACHE["nc"] = _build_bass()
    nc = _CACHE["nc"]
    in_maps = _prep_inputs(**inputs)
    res = run_bass_kernel_spmd(nc, in_maps, core_ids=list(range(N_CORES)))
    _CACHE["last_results"] = res
    acc = np.zeros((S, D_MODEL), np.float32)
    for c in range(N_CORES):
        acc += res.results[c]["out"].astype(np.float32)
    acc += inputs["wc_b"].astype(np.float32)[None, :]
    return acc[None, :, :]


if __name__ == "__main__":
    import reference

    inp = {k: np.asarray(v) for k, v in reference.setup_inputs().items()}
    got = kernel(**inp)
    print("kernel out", got.shape, got.dtype)
